# revision 34
# baseline (speedup 1.0000x reference)
"""GAT layer kernel for Trainium2, 8 NeuronCores, batch-sharded.

Math (per graph g of B=128, M=512 nodes, in=128, out D=64):
    Wh = h @ W.T;  s_src = Wh @ a[:D];  s_dst = Wh @ a[D:]
    e[i,j] = leakyrelu_0.2(s_src[i] + s_dst[j])
    out = elu(softmax(e, -1) @ Wh)

Key identity (this is what makes the kernel fast): exp is monotone, so
    exp(leakyrelu(e)) = max(exp(e), exp(e/5))
and e = s_src[m] + s_dst[n] makes both branches rank-1 separable.
Softmax is scale-invariant per column m, so the exp(s_src[m]) factor
cancels, leaving
    P'[n, m] = max(b1[n], q[m] * b2[n])
with b1 = exp(s_dst), b2 = exp(0.2*s_dst) (per-partition scalars) and
q[m] = exp(-0.8*s_src[m]). The 512x512 transcendental work of the
baseline (Prelu + Exp over e) collapses to ONE tensor_scalar per
128-chunk (two per-partition AP scalars: mult then max; measured
412 ns) plus one full-width exp.

q[m] is materialized replicated across partitions with the broadcast-
matmul trick: wsb is a host constant whose every column is -0.8*w_src,
so one matmul lhsT=wsb, rhs=ht gives qb[n, m] = -0.8*s_src[m] in PSUM,
and a single ACT Exp (FD-bound; partition count is free) yields qrep.

Per graph (21 matmuls): 1 wsb broadcast MM (first: it heads the
longest chain, qb -> qexp -> P'), 4 phase-1 MMs (rhs = wq =
[W.T | w_dst | 0.2*w_dst]), 16 attention MMs accumulating p_o [m, 65]
(ones column -> softmax denominator Z). Finale:
elu(x) = min(exp(x) - 1, relu(x)), x = p_o * (1/Z) in bf16.

Engine balance per graph (~ns): DVE = 4x412 P' + ~120 fast-recip +
419 x + ~310 stt (paired); ACT = 720 qexp + 300 bcol + paired elu
exp/relu (~720 each per PAIR) + 472 wha copy; PE = 21 MMs (~1.4us);
sync = 2 fat-line DMAs. The loop is software-pipelined: the finale of
graph pair (g-3, g-2) is emitted inside iteration g (odd g), with x
from both graphs landing in one shared tile so the elu exp/relu/stt
run as single FD=512 ops (per-op overhead amortized); next graph's
front matmuls are hoisted ahead of this graph's attention on PE; DMA
issue is split across the sync and gpsimd queues. Host pre-permutes
node order (pi(i) = 4*(i%128) + i//128) so the output tile
[128p, 4mc, 64] DMAs with 1KB contiguous lines and reshapes to node
order on host.
"""

import os
import sys
import types
from contextlib import ExitStack

import numpy as np
import ml_dtypes

# Defensive: concourse.bass_utils imports antenv.axon_hooks when tracing is
# requested (BASS_TRACE). Some images lack that module; register a stub so a
# traced run degrades to untraced instead of crashing.
try:
    import antenv.axon_hooks  # noqa: F401
except Exception:
    try:
        import antenv

        _hooks = types.ModuleType("antenv.axon_hooks")
        _hooks._hook = None
        _hooks.set_axon_ntff_profile_hook = lambda h: setattr(_hooks, "_hook", h)
        _hooks.get_axon_ntff_profile_hook = lambda: _hooks._hook
        sys.modules["antenv.axon_hooks"] = _hooks
        antenv.axon_hooks = _hooks
    except Exception:
        pass

import concourse.bass as bass
import concourse.tile as tile
from concourse import bacc, mybir
from concourse._compat import with_exitstack
from concourse.bass import ds, ts
from concourse.bass_utils import run_bass_kernel_spmd

B, M, IN_DIM, D = 128, 512, 128, 64
N_CORES = 8
G = B // N_CORES  # graphs per core
NC = M // 128  # 128-node chunks per graph
ALPHA = 0.2
F32 = mybir.dt.float32
BF16 = mybir.dt.bfloat16

LAST_RESULTS = None  # BassKernelResults of the most recent run (for test.py)


@with_exitstack
def _gat_body(ctx: ExitStack, tc: tile.TileContext, out_ap, ht_ap, wq_ap, wsb_ap):
    nc = tc.nc
    const = ctx.enter_context(tc.tile_pool(name="const", bufs=1))
    ht_pool = ctx.enter_context(tc.tile_pool(name="ht", bufs=5))
    wha_pool = ctx.enter_context(tc.tile_pool(name="wha", bufs=4))
    sc_pool = ctx.enter_context(tc.tile_pool(name="sc", bufs=4))
    q_pool = ctx.enter_context(tc.tile_pool(name="q", bufs=4))
    p_pool = ctx.enter_context(tc.tile_pool(name="p", bufs=4))
    fin_pool = ctx.enter_context(tc.tile_pool(name="fin", bufs=3))
    out_pool = ctx.enter_context(tc.tile_pool(name="out", bufs=3))
    ps_wh = ctx.enter_context(tc.tile_pool(name="ps_wh", bufs=2, space="PSUM"))
    ps_qb = ctx.enter_context(tc.tile_pool(name="ps_qb", bufs=2, space="PSUM"))
    ps_o = ctx.enter_context(tc.tile_pool(name="ps_o", bufs=4, space="PSUM"))

    # wsb gates the very first matmul (qb), so it heads the sync queue --
    # the earliest-ready DMA path; wq (needed slightly later) goes on gpsimd
    wsb_s = const.tile([IN_DIM, 128], BF16)
    nc.sync.dma_start(wsb_s[:], wsb_ap[:])
    wq_s = const.tile([IN_DIM, D + 2], BF16)
    nc.gpsimd.dma_start(wq_s[:], wq_ap[:])

    # Dummy activation at kernel start: triggers the one-time ACT table
    # load while the first ht DMA is in flight. memzero (Copy, scale=0) is
    # issued on ACT itself so the table load doesn't wait on another engine.
    warm = const.tile([1, 16], F32)
    nc.scalar.memzero(warm[:])
    nc.scalar.activation(warm[:], warm[:], mybir.ActivationFunctionType.Exp)

    fin = {}  # finale state: per-graph p_o, per-pair shared x2/e2/r2

    def fin_dve(g):
        """recip + x for graph g; x lands in the pair-shared x2 tile so the
        pair's E/R/stt run as single FD=512 ops (overhead amortized)."""
        pk, half = g // 2, g % 2
        p_o = fin[g, "p_o"]
        r4 = fin_pool.tile([128, NC], F32, tag="r4")
        nc.vector.reciprocal_approx_fast(r4[:], p_o[:, :, D])
        if (pk, "x2") not in fin:
            x2_new = fin_pool.tile([128, 2 * NC, D], BF16, tag="x")
            fin[pk, "x2"] = x2_new
        x2 = fin[pk, "x2"]
        r4b = r4[:].unsqueeze(2).broadcast_to([128, NC, D])
        nc.vector.tensor_tensor(
            x2[:, half * NC : (half + 1) * NC, :],
            p_o[:, :, 0:D], r4b, mybir.AluOpType.mult,
        )

    def fin_act2(pk):
        """elu pieces for a graph pair on ACT: E = exp(x), R = relu(x)."""
        x2 = fin[pk, "x2"]
        e2 = fin_pool.tile([128, 2 * NC, D], F32, tag="e")
        nc.scalar.activation(e2[:], x2[:], mybir.ActivationFunctionType.Exp)
        r2 = fin_pool.tile([128, 2 * NC, D], F32, tag="r")
        nc.scalar.activation(r2[:], x2[:], mybir.ActivationFunctionType.Relu)
        fin[pk, "er"] = (e2, r2)

    def fin_tail2(pk):
        """elu(x) = min(exp(x) - 1, relu(x)) for the pair + per-graph DMA."""
        e2, r2 = fin.pop((pk, "er"))
        fin.pop((pk, "x2"))
        o2 = out_pool.tile([128, 2 * NC, D], F32)
        nc.vector.scalar_tensor_tensor(
            o2[:], e2[:], -1.0, r2[:],
            mybir.AluOpType.add, mybir.AluOpType.min,
        )
        for half in range(2):
            g = 2 * pk + half
            fin.pop((g, "p_o"))
            nc.sync.dma_start(
                out_ap[g], o2[:, half * NC : (half + 1) * NC, :]
            )

    def fin_single(g, lo, hi, last=False):
        """Unpaired finale on an mc slice -- used only for the final two
        graphs, interleaved with the last attention so the drain overlaps."""
        p_o = fin[g, "p_o"]
        w = hi - lo
        r4 = fin_pool.tile([128, NC], F32, tag="r4")
        nc.vector.reciprocal_approx_fast(r4[:, 0:w], p_o[:, lo:hi, D])
        x_t = fin_pool.tile([128, NC, D], BF16, tag="x1")
        r4b = r4[:, 0:w].unsqueeze(2).broadcast_to([128, w, D])
        nc.vector.tensor_tensor(
            x_t[:, 0:w, :], p_o[:, lo:hi, 0:D], r4b, mybir.AluOpType.mult
        )
        e_t = fin_pool.tile([128, NC, D], F32, tag="e1")
        nc.scalar.activation(
            e_t[:, 0:w, :], x_t[:, 0:w, :], mybir.ActivationFunctionType.Exp
        )
        r_t = fin_pool.tile([128, NC, D], F32, tag="r1")
        nc.scalar.activation(
            r_t[:, 0:w, :], x_t[:, 0:w, :], mybir.ActivationFunctionType.Relu
        )
        o_t = out_pool.tile([128, NC, D], F32, tag="o1")
        nc.vector.scalar_tensor_tensor(
            o_t[:, 0:w, :], e_t[:, 0:w, :], -1.0, r_t[:, 0:w, :],
            mybir.AluOpType.add, mybir.AluOpType.min,
        )
        nc.sync.dma_start(out_ap[g, :, lo:hi, :], o_t[:, 0:w, :])
        if last:
            fin.pop((g, "p_o"))

    ht_tiles = {}

    def fetch_ht(g):
        t = ht_pool.tile([IN_DIM, M], BF16)
        nc.sync.dma_start(t[:], ht_ap[g])
        ht_tiles[g] = t

    mm_state = {}

    def front_mms(g):
        """qb + phase-1 matmuls for graph g. Emitted ahead of graph g-1's
        attention in the PE queue so the g exps never wait on the g-1
        attention drain."""
        ht_s = ht_tiles.pop(g)
        # qb[n, m] = -0.8*s_src[m] (all-columns-equal wsb): first, since it
        # heads the longest chain (qb -> qexp -> P' -> attention)
        qb_ps = ps_qb.tile([128, M], F32)
        nc.tensor.matmul(qb_ps[:], wsb_s[:], ht_s[:], start=True, stop=True)
        # Wh + score columns per node chunk: psum [128n, 66]
        p_wh = ps_wh.tile([128, NC, D + 2], F32)
        for c in range(NC):
            nc.tensor.matmul(
                p_wh[:, c, :], ht_s[:, ts(c, 128)], wq_s[:], start=True, stop=True
            )
        mm_state[g] = (qb_ps, p_wh)

    fetch_ht(0)
    fetch_ht(1)
    front_mms(0)
    for g in range(G):
        # prefetch ht two graphs ahead (front_mms(g+1) runs this iteration)
        if g + 2 < G:
            fetch_ht(g + 2)
        qb_ps, p_wh = mm_state.pop(g)

        # qrep[n, m] = exp(-0.8*s_src[m]) -- heads ACT queue each iteration
        qrep = q_pool.tile([128, M], BF16)
        nc.scalar.activation(qrep[:], qb_ps[:], mybir.ActivationFunctionType.Exp)
        # b1 = exp(s_dst), b2 = exp(0.2*s_dst) (per-partition fp32 scalars)
        bcol = sc_pool.tile([128, NC, 2], F32)
        nc.scalar.activation(
            bcol[:], p_wh[:, :, D : D + 2], mybir.ActivationFunctionType.Exp
        )

        # pair finale DVE parts for (g-3, g-2) at odd iterations (attention
        # long finished -- these fill DVE while P' waits on qrep)
        if g >= 3 and g % 2 == 1:
            fin_dve(g - 3)
            fin_dve(g - 2)

        # wha [n, 65] = [Wh | 1] bf16 (ones column -> softmax denominator).
        # Emitted BEFORE the pair's E2/R2 on ACT: wha gates this graph's
        # attention and must not queue behind the finale's x2 wait.
        wha = wha_pool.tile([128, NC, D + 1], BF16)
        nc.scalar.activation(
            wha[:, :, 0:D], p_wh[:, :, 0:D], mybir.ActivationFunctionType.Copy
        )
        nc.gpsimd.memset(wha[:, :, D : D + 1], 1.0)

        if g >= 3 and g % 2 == 1:
            fin_act2((g - 3) // 2)

        # P'[n, m] = max(q[m]*b2[n], b1[n]): one tensor_scalar per chunk
        p1 = p_pool.tile([128, NC, M], BF16)
        for c in range(NC):
            nc.vector.tensor_scalar(
                p1[:, c, :],
                qrep[:],
                bcol[:, c, 1:2],
                bcol[:, c, 0:1],
                mybir.AluOpType.mult,
                mybir.AluOpType.max,
            )

        # pair stt + out-DMA after P' on the DVE queue (its E2/R2 deps
        # resolve later than qrep; must not head-block P')
        if g >= 3 and g % 2 == 1:
            fin_tail2((g - 3) // 2)

        # next graph's front matmuls go ahead of this attention on PE
        if g + 1 < G:
            front_mms(g + 1)

        # attention: psum [m, 65]; col 64 = Z_m. The last two graphs'
        # finales are unpaired and interleaved with the final attention MMs
        # so the kernel tail overlaps instead of draining serially.
        p_o = ps_o.tile([128, NC, D + 1], F32)
        fin[g, "p_o"] = p_o
        if g == G - 1:
            fin_single(G - 2, 0, NC, last=True)
        for mc in range(NC):
            for c in range(NC):
                nc.tensor.matmul(
                    p_o[:, mc, :],
                    p1[:, c, ds(mc * 128, 128)],
                    wha[:, c, :],
                    start=(c == 0),
                    stop=(c == NC - 1),
                )
            if g == G - 1 and mc == 1:
                fin_single(g, 0, 2)
        if g == G - 1:
            fin_single(g, 2, NC, last=True)


_CACHE = {}


def _build():
    if "nc" in _CACHE:
        return _CACHE["nc"]
    nc = bacc.Bacc(
        "TRN2", target_bir_lowering=False, debug=False, num_devices=N_CORES
    )
    ht_d = nc.dram_tensor("ht", [G, IN_DIM, M], BF16, kind="ExternalInput")
    wq_d = nc.dram_tensor("wq", [IN_DIM, D + 2], BF16, kind="ExternalInput")
    wsb_d = nc.dram_tensor("wsb", [IN_DIM, 128], BF16, kind="ExternalInput")
    # out[g, p, mc, :] = node 4*p + mc  (host reshape restores node order)
    out_d = nc.dram_tensor("out", [G, 128, NC, D], F32, kind="ExternalOutput")
    with tile.TileContext(nc) as tc:
        _gat_body(tc, out_d.ap(), ht_d.ap(), wq_d.ap(), wsb_d.ap())
    nc.compile()
    _CACHE["nc"] = nc
    return nc


# Device column i holds node pi(i) = 4*(i % 128) + i // 128, so that the
# attention output tile [128p, 4mc, D] is node-ordered after a host reshape
# (node = 4p + mc) and the output DMA has 1KB-contiguous lines.
_PERM = (np.arange(M) % 128) * NC + (np.arange(M) // 128)


def host_prep(h, W, a):
    wt = W.T.astype(np.float32)  # [128, 64]
    w_src = wt @ a[:D]
    w_dst = wt @ a[D:]
    wq = np.concatenate(
        [wt, w_dst[:, None], 0.2 * w_dst[:, None]], axis=1
    ).astype(ml_dtypes.bfloat16)  # [128, 66]
    wsb = np.ascontiguousarray(
        np.repeat((-0.8 * w_src)[:, None], 128, axis=1)
    ).astype(ml_dtypes.bfloat16)  # [128, 128], every column -0.8*w_src
    return wq, wsb


def kernel(h, W, a):
    global LAST_RESULTS
    h = np.asarray(h, dtype=np.float32)
    W = np.asarray(W, dtype=np.float32)
    a = np.asarray(a, dtype=np.float32)

    wq, wsb = host_prep(h, W, a)

    nc = _build()
    in_maps = []
    for c in range(N_CORES):
        h_c = h[c * G : (c + 1) * G]  # [G, 512, 128]
        ht_c = np.ascontiguousarray(
            h_c[:, _PERM, :].transpose(0, 2, 1)
        ).astype(ml_dtypes.bfloat16)  # [G, 128, 512]
        in_maps.append({"ht": ht_c, "wq": wq, "wsb": wsb})

    res = run_bass_kernel_spmd(nc, in_maps, list(range(N_CORES)))
    LAST_RESULTS = res
    out = np.concatenate(
        [np.asarray(r["out"], dtype=np.float32).reshape(G, M, D) for r in res.results],
        axis=0,
    )
    return out.astype(np.float32)
